# revision 25
# baseline (speedup 1.0000x reference)
"""CGCConv-style GNN message passing kernel for 8 Trainium2 NeuronCores.

Reference computation (per edge e: src j -> dst i):
    msgs = edge_weight[:, None] * x[src] * pagerank[src][:, None]      # [E, D]
    aggr = segment_sum(msgs, dst, N)                                    # [N, D]
    out  = (aggr + x) @ W.T + b                                         # [N, D]

Strategy (edge-parallel by destination-node window; no collectives):
  - Nodes padded to 50176 = 392 windows of 128. Windows are assigned to
    (core, slot) pairs sorted by edge count so that the 8 cores see nearly
    identical per-bucket tile counts (one SPMD program, minimal padding).
  - Gather table xpi[25088, 256] fp16 interleaves the two halves of the
    node space: row r = [x*pr][r] (cols 0:96) and [x*pr][r+25088]
    (cols 128:224).  512-byte rows keep the DMA at full bandwidth and the
    row index fits int16.  Per edge we gather one row and select the
    correct half with half-masked one-hots (two matmuls per tile).
  - Edges bucketed per (slot, 64-dst sub-window); bucket sizes are the max
    across cores (no 128-rounding).  Buckets start mid-tile: a tile spanning
    two buckets gets one "build" per bucket, with foreign slots masked by
    zeros the host bakes into the per-build cmb table columns.
  - Device: per chunk of ~3 windows: dma_gather a tile range; build the
    eq-one-hot on DVE (iota == drel) over the chunk's builds, scale by
    cmb_lo (DVE) and cmb_hi (split ~75/25 GpSimd/DVE; all-DVE for the tail
    chunks so Pool is free for the last SWDGE generations); TensorE
    accumulates aggr.T[96,128] per window in PSUM (two matmuls per build).
    The gather index table loads compact [16, S/16] and is replicated to
    128 partitions on-device by an exact one-hot f32 matmul (8x fewer DMA
    bytes); index chains are prefetched two chunks ahead so SWDGE
    descriptor generation overlaps the previous gather.  The x-residual
    table loads in two halves inside the loop so the first gather is not
    queued behind const DMA.  Steady state runs gap-free at the exclusive
    DMA-engine roofline (~154us busy of ~165us total).
  - Final linear per window: out = [aggrT;ones] @ [W.T;b] + xT @ W.T via
    two accumulating matmuls; fp16 result staged and DMAd per chunk.
"""

import sys

for _p in ("/opt/trn_rl_repo",):
    if _p not in sys.path:
        sys.path.insert(0, _p)

import numpy as np

import concourse.mybir as mybir
import concourse.tile as tile
from concourse import bacc
from concourse.bass_utils import run_bass_kernel_spmd

F32 = mybir.dt.float32
F16 = mybir.dt.float16
I16 = mybir.dt.int16

N_NODES = 50000
D = 96
NCORES = 8
WIN = 128          # nodes per PSUM window
SUB = 64           # one-hot width (64-node sub-window)
NW = 49            # windows (slots) per core
PER = WIN * NW     # 6272 nodes per core
NPAD = PER * NCORES  # 50176
HALF = NPAD // 2   # 25088 rows in the interleaved table
ROWW = 256         # fp16 elems per table row (512 B)
CHUNK = 3          # windows per gather chunk

_LAST = {}         # debug/profiling stash: last built nc + run stats


def _host_prep(edge_index, edge_weight):
    """Balanced window assignment + bucketed slot tables per core."""
    src = np.asarray(edge_index[0], dtype=np.int64)
    dst = np.asarray(edge_index[1], dtype=np.int64)
    ew = np.asarray(edge_weight, dtype=np.float32)
    E = src.shape[0]

    wg = dst // WIN                                   # global window 0..391
    sub_g = (dst % WIN) // SUB
    cnt2 = np.zeros((NW * NCORES, 2), np.int64)
    np.add.at(cnt2, (wg, sub_g), 1)
    wtot = cnt2.sum(axis=1)
    order = np.argsort(-wtot, kind="stable")          # [392] window for (slot, core)

    # hill-climb the window->(slot,core) assignment to minimize total tiles
    def slot_cost(o, s):
        c = cnt2[o[s * NCORES : (s + 1) * NCORES]]
        return int(np.maximum(c.max(axis=0), 1).sum())

    costs = np.array([slot_cost(order, s) for s in range(NW)])
    rng = np.random.default_rng(0)
    ii = rng.integers(0, NW * NCORES, (60000, 2))
    for i, j in ii:
        si, sj = i // NCORES, j // NCORES
        if si == sj:
            continue
        order[i], order[j] = order[j], order[i]
        ci, cj = slot_cost(order, si), slot_cost(order, sj)
        if ci + cj <= costs[si] + costs[sj]:
            costs[si], costs[sj] = ci, cj
        else:
            order[i], order[j] = order[j], order[i]
    inv = np.empty_like(order)
    inv[order] = np.arange(order.size)

    pos = inv[wg]
    core = pos % NCORES
    slot_w = pos // NCORES                            # 0..48
    sub = (dst % WIN) // SUB
    b = slot_w * 2 + sub                              # bucket within core, 0..97
    g = core * (NW * 2) + b
    counts = np.bincount(g, minlength=NCORES * NW * 2).reshape(NCORES, NW * 2)

    # unpadded buckets: size = max across cores (padding only for core skew)
    sizes = np.maximum(1, counts.max(axis=0))                 # [98]
    off = np.zeros(NW * 2 + 1, dtype=np.int64)
    np.cumsum(sizes, out=off[1:])
    S_real = int(off[-1])
    T = (S_real + 127) // 128
    S = T * 128

    order_e = np.argsort(g, kind="stable")
    grp_counts = np.bincount(g, minlength=NCORES * NW * 2)
    starts = np.zeros(NCORES * NW * 2 + 1, dtype=np.int64)
    np.cumsum(grp_counts, out=starts[1:])
    rank = np.arange(E, dtype=np.int64) - starts[g[order_e]]
    slot = off[b[order_e]] + rank
    core_s = core[order_e]

    idx16 = np.zeros((NCORES, S), np.int16)
    c0 = np.zeros((NCORES, S), np.float16)
    c1 = np.zeros((NCORES, S), np.float16)
    dr = np.full((NCORES, S), -1.0, np.float16)
    src_o = src[order_e]
    hi = src_o >= HALF
    idx16[core_s, slot] = (src_o - hi * HALF).astype(np.int16)
    ewo = ew[order_e].astype(np.float16)
    c0[core_s, slot] = np.where(hi, np.float16(0), ewo)
    c1[core_s, slot] = np.where(hi, ewo, np.float16(0))
    dr[core_s, slot] = (dst[order_e] % SUB).astype(np.float16)

    # builds: one per (bucket, tile) pair the bucket spans
    b_lo = off[:-1] // 128
    b_hi = (off[1:] - 1) // 128
    nb = (b_hi - b_lo + 1).astype(np.int64)                   # [98]
    NB = int(nb.sum())
    build_bucket = np.repeat(np.arange(NW * 2), nb)
    build_tile = np.concatenate(
        [np.arange(b_lo[i], b_hi[i] + 1) for i in range(NW * 2)]
    )
    gslot = build_tile[None, :] * 128 + np.arange(128)[:, None]   # [128, NB]
    member = (gslot >= off[build_bucket][None, :]) & (
        gslot < off[build_bucket + 1][None, :]
    )
    c0_d = np.ascontiguousarray(c0[:, gslot] * member[None])      # [NC,128,NB]
    c1_d = np.ascontiguousarray(c1[:, gslot] * member[None])
    dr_d = np.ascontiguousarray(dr[:, gslot])

    idx_w = idx16.reshape(NCORES, S // 16, 16).transpose(0, 2, 1)
    idx_d = np.ascontiguousarray(idx_w)  # [NC,16,S/16]
    idx0_d = np.ascontiguousarray(np.tile(idx_w, (1, 8, 1)))  # [NC,128,S/16]

    return order, off, nb, build_tile, T, S, NB, idx_d, idx0_d, c0_d, c1_d, dr_d


def _build_nc(off, nb, build_tile, T, S, NB):
    """Single SPMD Bass program. off: [99] bucket slot offsets, nb: builds per
    bucket, build_tile: [NB] global tile of each build."""
    nc = bacc.Bacc(num_devices=NCORES)
    xpi_t = nc.dram_tensor("xpi", [HALF, ROWW], F16, kind="ExternalInput")
    wbt_t = nc.dram_tensor("wbt", [D + 1, D], F16, kind="ExternalInput")
    xt_t = nc.dram_tensor("xt", [D, PER], F16, kind="ExternalInput")
    io_t = nc.dram_tensor("io64", [128, SUB], F16, kind="ExternalInput")
    idx_t = nc.dram_tensor("idx", [16, S // 16], I16, kind="ExternalInput")
    rm_t = nc.dram_tensor("rmat", [16, 128], F32, kind="ExternalInput")
    on_t = nc.dram_tensor("onesrow", [1, PER], F16, kind="ExternalInput")
    idx0_t = nc.dram_tensor("idx0", [128, S // 16], I16, kind="ExternalInput")
    c0_t = nc.dram_tensor("c0", [128, NB], F16, kind="ExternalInput")
    c1_t = nc.dram_tensor("c1", [128, NB], F16, kind="ExternalInput")
    dr_t = nc.dram_tensor("dr", [128, NB], F16, kind="ExternalInput")
    out_t = nc.dram_tensor("out", [128, NW * D], F16, kind="ExternalOutput")

    # static bucket -> build ranges
    nb_end = np.cumsum(nb)
    nb_start = nb_end - nb
    # chunks of CHUNK windows: (s0, s1, tile range, build range)
    sizes = [1, 2] + [CHUNK] * 14 + [2, 1, 1]
    assert sum(sizes) == NW
    chunks = []
    s0 = 0
    prev_t1 = 0
    for sz in sizes:
        s1 = s0 + sz
        t1 = T if s1 == NW else int((off[s1 * 2] + 127) // 128)
        t1 = max(t1, prev_t1)
        chunks.append(
            (s0, s1, prev_t1, t1, int(nb_start[s0 * 2]), int(nb_end[s1 * 2 - 1]))
        )
        prev_t1 = t1
        s0 = s1
    chunk_of_tile = np.zeros(T, dtype=np.int64)
    for k, (_, _, t0, t1, _, _) in enumerate(chunks):
        chunk_of_tile[t0:t1] = k

    with tile.TileContext(nc) as tc:
        from contextlib import ExitStack

        with ExitStack() as ctx:
            const = ctx.enter_context(tc.tile_pool(name="const", bufs=1))
            idxp = ctx.enter_context(tc.tile_pool(name="idxp", bufs=3))
            gp = ctx.enter_context(tc.tile_pool(name="gp", bufs=3))
            eqp = ctx.enter_context(tc.tile_pool(name="eqp", bufs=2))
            oh0p = ctx.enter_context(tc.tile_pool(name="oh0p", bufs=3))
            oh1p = ctx.enter_context(tc.tile_pool(name="oh1p", bufs=3))
            obp = ctx.enter_context(tc.tile_pool(name="obp", bufs=2))
            icp = ctx.enter_context(tc.tile_pool(name="icp", bufs=3))
            icf = ctx.enter_context(tc.tile_pool(name="icf", bufs=2))
            psw = ctx.enter_context(tc.tile_pool(name="psw", bufs=2, space="PSUM"))
            psi = ctx.enter_context(tc.tile_pool(name="psi", bufs=2, space="PSUM"))
            psr = ctx.enter_context(tc.tile_pool(name="psr", bufs=2, space="PSUM"))

            rmat = const.tile([16, 128], F32)
            nc.sync.dma_start(out=rmat[:, :], in_=rm_t[:, :])

            def load_idc(k):
                """Load [16, n] indices, replicate to [128, n] via an exact
                one-hot f32 matmul on the otherwise idle PE (saves 8x the
                DMA bytes of loading the replicated table from DRAM)."""
                s0, s1, ct0, ct1, nb0, nb1 = chunks[k]
                n = (ct1 - ct0) * 8
                if k < 0:  # direct idx0 path disabled — chains schedule best
                    idc = icp.tile([128, n], I16, tag="idc")
                    nc.sync.dma_start(
                        out=idc[:, :], in_=idx0_t[:, ct0 * 8 : ct1 * 8]
                    )
                    return idc
                i16 = idxp.tile([16, n], I16, tag="idc16")
                nc.sync.dma_start(out=i16[:, :], in_=idx_t[:, ct0 * 8 : ct1 * 8])
                iflt = icf.tile([16, n], F32, tag="icf")
                nc.scalar.copy(out=iflt[:, :], in_=i16[:, :])
                idc = icp.tile([128, n], I16, tag="idc")
                h = n if n <= 512 else (n + 1) // 2
                for a, b in (((0, n),) if n <= 512 else ((0, h), (h, n))):
                    pi = psi.tile([128, h], F32, tag="pi")
                    nc.tensor.matmul(
                        out=pi[:, : b - a],
                        lhsT=rmat[:, :],
                        rhs=iflt[:, a:b],
                        start=True,
                        stop=True,
                        skip_group_check=True,
                    )
                    nc.scalar.copy(out=idc[:, a:b], in_=pi[:, : b - a])
                return idc

            def gather(k, idc):
                s0, s1, ct0, ct1, nb0, nb1 = chunks[k]
                m = ct1 - ct0
                gt = gp.tile([128, m, ROWW], F16, tag="g")
                nc.gpsimd.dma_gather(
                    out_ap=gt[:, :, :],
                    in_ap=xpi_t[:, :],
                    idxs_ap=idc[:, :],
                    num_idxs=m * 128,
                    num_idxs_reg=m * 128,
                    elem_size=ROWW,
                    single_packet=False,
                )
                return gt

            # prefetch indices two chunks ahead so SWDGE desc-gen for chunk
            # k+1 overlaps the chunk-k gather transfer (keeps DMA saturated)
            idcs = [load_idc(0)]
            gts = {0: gather(0, idcs[0])}
            idcs.append(load_idc(1))

            # constants load in the DMA shadow of the first SWDGE gen
            c0r = const.tile([128, NB], F16)
            nc.sync.dma_start(out=c0r[:, :], in_=c0_t[:, :])
            c1r = const.tile([128, NB], F16)
            nc.sync.dma_start(out=c1r[:, :], in_=c1_t[:, :])
            drr = const.tile([128, NB], F16)
            nc.sync.dma_start(out=drr[:, :], in_=dr_t[:, :])
            iota = const.tile([128, SUB], F16)
            nc.sync.dma_start(out=iota[:, :], in_=io_t[:, :])
            wbt = const.tile([D + 1, D], F16)
            nc.sync.dma_start(out=wbt[:, :], in_=wbt_t[:, :])
            # xtr halves load inside the loop so the first gather is not
            # queued behind 3.3us of const DMA
            xtr = const.tile([D, PER], F16)

            # aggr.T with a trailing ones-row (bias via the final matmul)
            aggrT = const.tile([D + 1, PER], F16)
            nc.sync.dma_start(out=aggrT[D : D + 1, :], in_=on_t[:, :])

            for k, (s0, s1, ct0, ct1, nb0, nb1) in enumerate(chunks):
                m = nb1 - nb0
                gt = gts.pop(k) if k in gts else gather(k, idcs[k])
                gts[k] = gt
                if k + 2 < len(chunks):
                    idcs.append(load_idc(k + 2))
                if k in (0, 2):
                    h0 = 0 if k == 0 else PER // 2
                    h1 = PER // 2 if k == 0 else PER
                    nc.sync.dma_start(
                        out=xtr[:, h0:h1], in_=xt_t[:, h0:h1]
                    )
                eq = eqp.tile([128, m, SUB], F16, tag="eq")
                nc.vector.tensor_tensor(
                    out=eq[:, :, :],
                    in0=iota[:, None, :].to_broadcast([128, m, SUB]),
                    in1=drr[:, nb0:nb1, None].to_broadcast([128, m, SUB]),
                    op=mybir.AluOpType.is_equal,
                )
                oh0 = oh0p.tile([128, m, SUB], F16, tag="oh0")
                nc.vector.tensor_tensor(
                    out=oh0[:, :, :],
                    in0=eq[:, :, :],
                    in1=c0r[:, nb0:nb1, None].to_broadcast([128, m, SUB]),
                    op=mybir.AluOpType.mult,
                )
                oh1 = oh1p.tile([128, m, SUB], F16, tag="oh1")
                # split the third one-hot pass ~75/25 between GpSimd and DVE;
                # tail chunks go all-DVE so Pool is free for the last SWDGEs
                if k >= len(chunks) - 3:
                    mp = 0
                else:
                    mp = max(1, (3 * m) // 4) if m > 1 else m
                if mp > 0:
                    nc.gpsimd.tensor_tensor(
                        out=oh1[:, 0:mp, :],
                        in0=eq[:, 0:mp, :],
                        in1=c1r[:, nb0 : nb0 + mp, None].to_broadcast(
                            [128, mp, SUB]
                        ),
                        op=mybir.AluOpType.mult,
                    )
                if mp < m:
                    nc.vector.tensor_tensor(
                        out=oh1[:, mp:m, :],
                        in0=eq[:, mp:m, :],
                        in1=c1r[:, nb0 + mp : nb1, None].to_broadcast(
                            [128, m - mp, SUB]
                        ),
                        op=mybir.AluOpType.mult,
                    )

                ob = obp.tile([128, (s1 - s0) * D], F16, tag="ob")
                for s in range(s0, s1):
                    ps = psw.tile([D, WIN], F32, tag="ps")
                    for sg in (0, 1):
                        a0 = int(nb_start[s * 2 + sg])
                        a1 = int(nb_end[s * 2 + sg])
                        for j in range(a0, a1):
                            t = int(build_tile[j])
                            kc = int(chunk_of_tile[t])
                            gk = gts[kc]
                            jj = t - chunks[kc][2]
                            nc.tensor.matmul(
                                out=ps[:, sg * SUB : (sg + 1) * SUB],
                                lhsT=gk[:, jj, 0:D],
                                rhs=oh0[:, j - nb0, :],
                                start=(j == a0),
                                stop=False,
                                skip_group_check=True,
                            )
                            nc.tensor.matmul(
                                out=ps[:, sg * SUB : (sg + 1) * SUB],
                                lhsT=gk[:, jj, 128 : 128 + D],
                                rhs=oh1[:, j - nb0, :],
                                start=False,
                                stop=(j == a1 - 1),
                                skip_group_check=True,
                            )
                    nc.scalar.copy(
                        out=aggrT[:D, s * WIN : (s + 1) * WIN], in_=ps[:, :]
                    )
                    rp = psr.tile([WIN, D], F32, tag="rp")
                    nc.tensor.matmul(
                        out=rp[:, :],
                        lhsT=aggrT[:, s * WIN : (s + 1) * WIN],
                        rhs=wbt[:, :],
                        start=True,
                        stop=False,
                        skip_group_check=True,
                    )
                    nc.tensor.matmul(
                        out=rp[:, :],
                        lhsT=xtr[:, s * WIN : (s + 1) * WIN],
                        rhs=wbt[:D, :],
                        start=False,
                        stop=True,
                        skip_group_check=True,
                    )
                    nc.scalar.copy(out=ob[:, (s - s0) * D : (s - s0 + 1) * D], in_=rp[:, :])
                nc.sync.dma_start(out=out_t[:, s0 * D : s1 * D], in_=ob[:, :])

    nc.compile()
    return nc


def kernel(x, edge_index, edge_weight, pagerank, W, b):
    x = np.asarray(x, np.float32)
    pr = np.asarray(pagerank, np.float32)
    W = np.asarray(W, np.float32)
    b = np.asarray(b, np.float32)

    order, off, nb, build_tile, T, S, NB, idx_d, idx0_d, c0_d, c1_d, dr_d = _host_prep(
        edge_index, edge_weight
    )

    xpr = np.zeros((NPAD, D), np.float32)
    xpr[:N_NODES] = x * pr[:, None]
    xpi = np.zeros((HALF, ROWW), np.float16)
    xpi[:, 0:D] = xpr[:HALF]
    xpi[:, 128 : 128 + D] = xpr[HALF:]

    xpad = np.zeros((NPAD, D), np.float32)
    xpad[:N_NODES] = x

    wbt = np.zeros((D + 1, D), np.float16)
    wbt[:D] = W.T.astype(np.float16)
    wbt[D] = b.astype(np.float16)

    io64 = np.broadcast_to(np.arange(SUB, dtype=np.float16), (128, SUB)).copy()
    rmat = (np.arange(128)[None, :] % 16 == np.arange(16)[:, None]).astype(np.float32)
    onesrow = np.ones((1, PER), np.float16)

    nc = _build_nc(off, nb, build_tile, T, S, NB)

    in_maps = []
    for c in range(NCORES):
        wins = order[np.arange(NW) * NCORES + c]          # global window per slot
        xt = np.ascontiguousarray(
            xpad.reshape(NPAD // WIN, WIN, D)[wins]        # [NW,128,D]
            .transpose(2, 0, 1).reshape(D, PER)            # [D, NW*128]
        ).astype(np.float16)
        in_maps.append(
            {
                "xpi": xpi,
                "wbt": wbt,
                "xt": xt,
                "io64": io64,
                "idx": idx_d[c],
                "idx0": idx0_d[c],
                "rmat": rmat,
                "onesrow": onesrow,
                "c0": c0_d[c],
                "c1": c1_d[c],
                "dr": dr_d[c],
            }
        )
    import time

    t0 = time.time()
    res = run_bass_kernel_spmd(nc, in_maps, core_ids=list(range(NCORES)))
    _LAST.update(nc=nc, run_wall_s=time.time() - t0)

    out = np.zeros((NPAD, D), np.float32)
    for c in range(NCORES):
        ob = np.asarray(res.results[c]["out"], np.float32)  # [128, NW*D]
        wins = order[np.arange(NW) * NCORES + c]
        out[wins[:, None] * WIN + np.arange(WIN)[None, :]] = (
            ob.reshape(128, NW, D).transpose(1, 0, 2)
        )
    return out[:N_NODES]
